# revision 14
# baseline (speedup 1.0000x reference)
"""Policy-masked multi-head attention block (ViT-style) on 8 TRN2 NeuronCores.

Sharding: data-parallel over batch. B=16 -> 2 batches per core, no collectives.

Key optimization vs naive: tokens are permuted host-side so kept tokens
(policy=1) come first. Kept count m ~ Binomial(577, .5) <= 384 w.p.
1-1e-15, so attention KEYS are compacted to the first 384 tokens (3 tiles
of 128 instead of 5): logits, exp and AV shrink by 40%. Dropped queries
(q >= m) keep their self-attention term:
  - q in [m, 384): self is among the (masked) keys; restored by a
    per-tile diagonal max  E <- max(E, ident * d)  (d = exp(SCALE*q.k)).
  - q in [384, 577): handled by two extra AV accumulation steps with a
    diagonal moving matrix D_t = ident * d (also supplies the ones-row
    denominator contribution).
The output is un-permuted host-side.

Math (per batch, matches reference up to O(1e-3) in bf16):
  qkv = x @ Wqkv + bqkv ; q,k,v per head (Dh=64)
  E[j,q] = exp(SCALE*z[j,q] + mb_j)   (mb_j = -1e4 for dropped keys)
  num[d,q] = sum_j v[j,d]*E[j,q]  (+ diag terms)   den[q] = ones-row
  attn_out[q,d] = num[d,q]/den[q] ;  out = attn_out @ Wproj + bproj
No max-subtraction: |SCALE*z| is O(1) for this input distribution.

Layout: keys-on-partitions / queries-on-free; the policy mask is a
per-partition activation bias; the key-sum (denominator) rides the AV
matmul as a ones column of V; E feeds the AV matmul directly as the
moving operand. Even/odd heads of a pair are processed interleaved
(separate PSUM accumulators) so the PE never waits for a deferred pass.
"""

import os
import ml_dtypes
import numpy as np

import concourse.bass as bass
import concourse.bacc as bacc
import concourse.mybir as mybir
import concourse.tile as tile
from concourse.masks import make_identity

# problem constants (hardcoded per contract)
B = 16
N = 577
C = 768
H = 12
DH = 64
SCALE = DH ** -0.5
EPS = 1e-6
NCORES = 8
BB = B // NCORES          # batches per core
NPAD = 640                # tokens padded to 5*128
NT = NPAD // 128          # 5 token tiles
NKT = 3                   # compacted key tiles (384 keys)
MKEY = NKT * 128          # 384
NF = C // 128             # 6 feature tiles
F32 = mybir.dt.float32
BF16 = mybir.dt.bfloat16
DT = BF16
SCALE_EXP = SCALE

MASK_NEG = -1.0e4
Q1, Q2 = 320, N - 320     # query chunks for 577-wide streams (QK proj)


def build_kernel():
    nc = bacc.Bacc()

    x_d = nc.declare_dram_parameter("x", [BB, NF, 128, NPAD], DT, isOutput=False)
    mb_d = nc.declare_dram_parameter("maskb", [BB, MKEY], F32, isOutput=False)
    wqkv_d = nc.declare_dram_parameter("wqkv", [C, 3 * C], DT, isOutput=False)
    wproj_d = nc.declare_dram_parameter("wproj", [C, C], DT, isOutput=False)
    bqkv_d = nc.declare_dram_parameter("bqkv", [3 * C], F32, isOutput=False)
    bproj_d = nc.declare_dram_parameter("bproj", [C], F32, isOutput=False)
    onehd_d = nc.declare_dram_parameter("ones_hd", [C, H], DT, isOutput=False)
    out_d = nc.declare_dram_parameter("out", [BB * N, C], F32, isOutput=True)

    with tile.TileContext(nc) as tc:
        with (
            tc.tile_pool(name="singles", bufs=1) as singles,
            tc.tile_pool(name="pbatch", bufs=2) as pb,
            tc.tile_pool(name="pe", bufs=2) as pe_pool,
            tc.tile_pool(name="pnum", bufs=2) as pnum,
            tc.tile_pool(name="psmall", bufs=4) as psmall,
            tc.tile_pool(name="ptrans", bufs=2) as ptrans,
            tc.tile_pool(name="pout", bufs=2) as pout,
            tc.tile_pool(name="ppA", bufs=2, space="PSUM") as ppA,
            tc.tile_pool(name="ppN", bufs=1, space="PSUM") as ppN,
        ):
            # ---- constants ----
            wqkv_sb = singles.tile([128, NF, 3 * C], DT)
            for f in range(NF):
                nc.sync.dma_start(
                    wqkv_sb[:, f, :],
                    wqkv_d.rearrange("(f p) m -> f p m", p=128)[f],
                )
            wproj_sb = singles.tile([128, NF, C], DT)
            for f in range(NF):
                nc.scalar.dma_start(
                    wproj_sb[:, f, :],
                    wproj_d.rearrange("(f p) m -> f p m", p=128)[f],
                )
            bqkv_sb = singles.tile([128, 2 * NF], F32)  # q,k feature bias chunks
            nc.scalar.dma_start(
                bqkv_sb[:, :],
                bqkv_d[0 : 2 * C].rearrange("(m p) -> p m", p=128),
            )
            # v-bias and proj-bias broadcast across all 128 partitions
            bv_bc = singles.tile([128, C], F32)
            nc.gpsimd.dma_start(
                out=bv_bc[:, :], in_=bqkv_d[2 * C : 3 * C].partition_broadcast(128)
            )
            bproj_bc = singles.tile([128, C], F32)
            nc.gpsimd.dma_start(
                out=bproj_bc[:, :], in_=bproj_d[:].partition_broadcast(128)
            )
            onehd_sb = singles.tile([128, NF, H], DT)
            nc.scalar.dma_start(
                onehd_sb[:, :, :],
                onehd_d.rearrange("(f p) h -> p f h", p=128),
            )
            ident_f32 = singles.tile([128, 128], F32)
            make_identity(nc, ident_f32)
            ident = singles.tile([128, 128], DT)
            nc.vector.tensor_copy(ident[:, :], ident_f32[:, :])
            ones60 = singles.tile([128, NT * H], F32)
            nc.vector.memset(ones60, 1.0)

            for b in range(BB):
                # ---- load mask + x (transposed to feature-major) ----
                mb_sb = psmall.tile([128, NKT], F32, tag="mb_sb")
                nc.sync.dma_start(
                    mb_sb[:, :], mb_d[b].rearrange("(t p) -> p t", p=128)
                )
                xT = pb.tile([128, NF, NPAD], DT, tag="xT")
                for f in range(NF):
                    nc.sync.dma_start(xT[:, f, :], x_d[b, f])

                # ---- QKV: q,k sections feature-major (577 cols only) ----
                qk_sb = pb.tile([128, 2 * NF, NPAD], DT, tag="qk_sb")
                nc.vector.memset(qk_sb[:, :, N:NPAD], 0.0)
                for m in range(2 * NF):
                    ps = ppA.tile([128, 2, 512], F32, tag="ppA")
                    for f in range(NF):
                        nc.tensor.matmul(
                            ps[:, 0, 0:Q1],
                            wqkv_sb[:, f, m * 128 : (m + 1) * 128],
                            xT[:, f, 0:Q1],
                            start=(f == 0),
                            stop=(f == NF - 1),
                        )
                        nc.tensor.matmul(
                            ps[:, 1, 0:Q2],
                            wqkv_sb[:, f, m * 128 : (m + 1) * 128],
                            xT[:, f, Q1:N],
                            start=(f == 0),
                            stop=(f == NF - 1),
                        )
                    if m % 2 == 0:
                        nc.vector.tensor_scalar_add(
                            qk_sb[:, m, 0:Q1], ps[:, 0, 0:Q1],
                            bqkv_sb[:, m : m + 1],
                        )
                        nc.vector.tensor_scalar_add(
                            qk_sb[:, m, Q1:N], ps[:, 1, 0:Q2],
                            bqkv_sb[:, m : m + 1],
                        )
                    else:
                        nc.scalar.activation(
                            qk_sb[:, m, 0:Q1], ps[:, 0, 0:Q1],
                            mybir.ActivationFunctionType.Identity,
                            bias=bqkv_sb[:, m : m + 1],
                        )
                        nc.scalar.activation(
                            qk_sb[:, m, Q1:N], ps[:, 1, 0:Q2],
                            mybir.ActivationFunctionType.Identity,
                            bias=bqkv_sb[:, m : m + 1],
                        )

                # ---- QKV: v section token-major, per-head layout, ones col
                v_sb = pb.tile([128, NT, H, DH + 1], DT, tag="v_sb")
                for t in range(NT):
                    ps = ppA.tile([128, 2, 512], F32, tag="ppA")
                    for f in range(NF):
                        for n0, n1 in ((0, 512), (512, 768)):
                            nc.tensor.matmul(
                                ps.rearrange("p s q -> p (s q)")[:, n0:n1],
                                xT[:, f, t * 128 : (t + 1) * 128],
                                wqkv_sb[:, f, 2 * C + n0 : 2 * C + n1],
                                start=(f == 0),
                                stop=(f == NF - 1),
                            )
                    nc.vector.tensor_tensor(
                        v_sb[:, t, :, 0:DH],
                        ps.rearrange("p s q -> p (s q)")[:, 0:C].rearrange(
                            "p (h d) -> p h d", h=H
                        ),
                        bv_bc.rearrange("p (h d) -> p h d", h=H),
                        mybir.AluOpType.add,
                    )
                nc.vector.tensor_copy(
                    v_sb[:, :, :, DH],
                    ones60.rearrange("p (t h) -> p t h", t=NT),
                )

                # ---- diagonal logits for all heads: d[h, j] = exp(s*q_j.k_j)
                psz = ppN.tile([128, 2, 512], F32, tag="nume")
                for f in range(NF):
                    qkel = ptrans.tile([128, N], DT, tag="qkel")
                    nc.vector.tensor_tensor(
                        qkel[:, :],
                        qk_sb[:, f, 0:N],
                        qk_sb[:, NF + f, 0:N],
                        mybir.AluOpType.mult,
                    )
                    nc.tensor.matmul(
                        psz[0:H, 0, 0:Q1],
                        onehd_sb[:, f, :],
                        qkel[:, 0:Q1],
                        start=(f == 0),
                        stop=(f == NF - 1),
                    )
                    nc.tensor.matmul(
                        psz[0:H, 1, 0:Q2],
                        onehd_sb[:, f, :],
                        qkel[:, Q1:N],
                        start=(f == 0),
                        stop=(f == NF - 1),
                    )
                d_all = psmall.tile([H, NPAD], F32, tag="d_all")
                nc.scalar.activation(
                    d_all[:, 0:Q1], psz[0:H, 0, 0:Q1],
                    mybir.ActivationFunctionType.Exp, scale=SCALE_EXP,
                )
                nc.scalar.activation(
                    d_all[:, Q1:N], psz[0:H, 1, 0:Q2],
                    mybir.ActivationFunctionType.Exp, scale=SCALE_EXP,
                )
                # keep pad region finite: D rows for pad tokens become 0
                nc.vector.memset(d_all[:, N:NPAD], 0.0)
                d_t = psmall.tile([128, NT, H], F32, tag="d_t")
                for t in range(NT):
                    pst_f = ppN.tile([128, 128], F32, tag="numo")
                    nc.tensor.transpose(
                        pst_f[:, 0:H],
                        d_all[:, t * 128 : (t + 1) * 128],
                        ident_f32[0:H, 0:H],
                    )
                    nc.vector.tensor_copy(d_t[:, t, :], pst_f[:, 0:H])

                # ---- attention, head pairs, software-pipelined emission ----
                # Per hp: logits(t)/exp(t) run one step ahead of AV(t); the
                # num->token-major tail of hp-1 is emitted inside hp's stream
                # so the PE never waits on the PSUM ring or cross-engine
                # copies. Diagonal D tiles are built lazily per hp.
                attn_sb = pb.tile([128, NT, H, DH], DT, tag="attn_sb")
                Dd = pb.tile([128, 2, H, 128], DT, tag="Dd")

                def emit_logits_exp(hp, t):
                    he, ho = 2 * hp, 2 * hp + 1
                    st_e = ppA.tile([128, 2, 512], F32, tag="ppA", name="st_e")
                    st_o = ppA.tile([128, 2, 512], F32, tag="ppA", name="st_o")
                    for s in range(2):
                        nc.tensor.matmul(
                            st_e[:, s, 0:320],
                            qk_sb[0:DH, NF + hp, t * 128 : (t + 1) * 128],
                            qk_sb[0:DH, hp, s * 320 : s * 320 + 320],
                            start=True,
                            stop=True,
                        )
                    for s in range(2):
                        nc.tensor.matmul(
                            st_o[:, s, 0:320],
                            qk_sb[DH:128, NF + hp, t * 128 : (t + 1) * 128],
                            qk_sb[DH:128, hp, s * 320 : s * 320 + 320],
                            start=True,
                            stop=True,
                        )
                    e_e = pe_pool.tile([128, NPAD], DT, tag="e_e", name="e_e")
                    e_o = pe_pool.tile([128, NPAD], DT, tag="e_o", name="e_o")
                    nc.scalar.activation(
                        e_e.rearrange("p (s q) -> p s q", s=2),
                        st_e[:, :, 0:320],
                        mybir.ActivationFunctionType.Exp,
                        bias=mb_sb[:, t : t + 1], scale=SCALE_EXP,
                    )
                    nc.scalar.activation(
                        e_o.rearrange("p (s q) -> p s q", s=2),
                        st_o[:, :, 0:320],
                        mybir.ActivationFunctionType.Exp,
                        bias=mb_sb[:, t : t + 1], scale=SCALE_EXP,
                    )
                    nc.vector.scalar_tensor_tensor(
                        out=e_e[:, t * 128 : (t + 1) * 128],
                        in0=ident,
                        scalar=d_t[:, t, he : he + 1],
                        in1=e_e[:, t * 128 : (t + 1) * 128],
                        op0=mybir.AluOpType.mult,
                        op1=mybir.AluOpType.max,
                    )
                    nc.vector.scalar_tensor_tensor(
                        out=e_o[:, t * 128 : (t + 1) * 128],
                        in0=ident,
                        scalar=d_t[:, t, ho : ho + 1],
                        in1=e_o[:, t * 128 : (t + 1) * 128],
                        op0=mybir.AluOpType.mult,
                        op1=mybir.AluOpType.max,
                    )
                    return e_e, e_o

                def emit_av(hp, t, nume, numo, e_e, e_o):
                    he, ho = 2 * hp, 2 * hp + 1
                    if t == NKT - 1:
                        # tail-query self terms: diagonal D tiles join the
                        # AV group before its final (stop) matmuls
                        for h, num in ((he, nume), (ho, numo)):
                            nc.tensor.matmul(
                                num[:, 1, 64:192],
                                v_sb[:, 3, h, :],
                                Dd[:, 0, h, :],
                                start=False, stop=False,
                                skip_group_check=True,
                            )
                            nc.tensor.matmul(
                                num[:, 1, 192:257],
                                v_sb[:, 4, h, :],
                                Dd[:, 1, h, 0:65],
                                start=False, stop=False,
                                skip_group_check=True,
                            )
                    for s in range(2):
                        nc.tensor.matmul(
                            nume[:, s, 0:320],
                            v_sb[:, t, he, :],
                            e_e[:, s * 320 : s * 320 + 320],
                            start=(t == 0),
                            stop=(t == NKT - 1),
                        )
                    for s in range(2):
                        nc.tensor.matmul(
                            numo[:, s, 0:320],
                            v_sb[:, t, ho, :],
                            e_o[:, s * 320 : s * 320 + 320],
                            start=(t == 0),
                            stop=(t == NKT - 1),
                        )

                def make_tail(hp, num_sbs):
                    def tail():
                        he, ho = 2 * hp, 2 * hp + 1
                        for par, h in ((0, he), (1, ho)):
                            num_sb = num_sbs[par]
                            nt_all = ppN.tile(
                                [128, NT, DH + 2], DT,
                                tag=("nume" if par == 0 else "numo"),
                                name="nt_all",
                            )
                            for t in range(NT):
                                nc.tensor.transpose(
                                    nt_all[:, t, 0 : DH + 1],
                                    num_sb[:, t * 128 : (t + 1) * 128],
                                    ident[0 : DH + 1, 0 : DH + 1],
                                )
                            r = psmall.tile([128, NT], F32, tag="r", name="r")
                            nc.vector.reciprocal(r[:, :], nt_all[:, :, DH])
                            nc.vector.tensor_tensor(
                                attn_sb[:, :, h, :],
                                nt_all[:, :, 0:DH],
                                r[:, :, None].to_broadcast([128, NT, DH]),
                                mybir.AluOpType.mult,
                            )
                    return tail

                aT_all = pb.tile([128, NT, NF, 128], DT, tag="aT_all")

                def emit_proj_transposes(f):
                    for t in range(NT):
                        pst = ppN.tile(
                            [128, 128], DT,
                            tag=("nume" if t % 2 == 0 else "numo"), name="pst",
                        )
                        nc.tensor.transpose(
                            pst[:, 0:128],
                            attn_sb.rearrange("p t h d -> p t (h d)")[
                                :, t, f * 128 : (f + 1) * 128
                            ],
                            ident,
                        )
                        if t % 2 == 0:
                            nc.scalar.activation(
                                aT_all[:, t, f, :], pst[:, 0:128],
                                mybir.ActivationFunctionType.Identity,
                            )
                        else:
                            nc.vector.tensor_copy(
                                aT_all[:, t, f, :], pst[:, 0:128]
                            )

                pending_tail = None
                for hp in range(H // 2):
                    he, ho = 2 * hp, 2 * hp + 1
                    e0 = emit_logits_exp(hp, 0)
                    e1 = emit_logits_exp(hp, 1)
                    if pending_tail is not None:
                        pending_tail()
                    # D tiles for this hp (DVE, queued behind the tail's
                    # normalize so AV(t0)'s ring dependency clears first)
                    for h in (he, ho):
                        for ti in range(2):
                            nc.vector.tensor_scalar_mul(
                                Dd[:, ti, h, :],
                                ident[:, :],
                                d_t[:, NKT + ti, h : h + 1],
                            )
                    nume = ppN.tile([DH + 1, 2, 512], F32, tag="nume")
                    numo = ppN.tile([DH + 1, 2, 512], F32, tag="numo")
                    emit_av(hp, 0, nume, numo, *e0)
                    if hp >= 1:
                        # transpose the previous hp's normalized attn columns
                        # for the projection while the PE stream is hot
                        emit_proj_transposes(hp - 1)
                    e2 = emit_logits_exp(hp, 2)
                    emit_av(hp, 1, nume, numo, *e1)
                    emit_av(hp, 2, nume, numo, *e2)
                    num_sbs = []
                    for par, num in ((0, nume), (1, numo)):
                        num_sb = pnum.tile(
                            [DH + 1, NPAD], DT, tag=f"num{par}", name="num_sb"
                        )
                        if par == 0:
                            nc.scalar.activation(
                                num_sb.rearrange("p (s q) -> p s q", s=2),
                                num[:, :, 0:320],
                                mybir.ActivationFunctionType.Identity,
                            )
                        else:
                            nc.vector.tensor_copy(
                                num_sb.rearrange("p (s q) -> p s q", s=2),
                                num[:, :, 0:320],
                            )
                        num_sbs.append(num_sb)
                    pending_tail = make_tail(hp, num_sbs)
                pending_tail()
                emit_proj_transposes(H // 2 - 1)

                # ---- proj: pure matmul stream from pre-transposed attn ----
                for t in range(NT):
                    pso = ppA.tile([128, 2, 512], F32, tag="ppA", name="pso")
                    for f in range(NF):
                        for n0, n1 in ((0, 512), (512, 768)):
                            nc.tensor.matmul(
                                pso.rearrange("p s q -> p (s q)")[:, n0:n1],
                                aT_all[:, t, f, :],
                                wproj_sb[:, f, n0:n1],
                                start=(f == 0),
                                stop=(f == NF - 1),
                            )
                    o_sb = pout.tile([128, C], F32, tag="o_sb", name="o_sb")
                    nc.vector.tensor_tensor(
                        o_sb[:, :],
                        pso.rearrange("p s q -> p (s q)")[:, 0:C],
                        bproj_bc[:, :],
                        mybir.AluOpType.add,
                    )
                    rows = 128 if t < NT - 1 else N - 4 * 128
                    nc.sync.dma_start(
                        out_d[b * N + t * 128 : b * N + t * 128 + rows, :],
                        o_sb[0:rows, :],
                    )
    nc.finalize()
    return nc


_NC_CACHE = None


def _get_nc():
    global _NC_CACHE
    if _NC_CACHE is None:
        _NC_CACHE = build_kernel()
    return _NC_CACHE


def _make_in_maps(x, policy, Wqkv, bqkv, Wproj, bproj):
    x = np.ascontiguousarray(np.asarray(x, dtype=np.float32))
    policy = np.asarray(policy, dtype=np.float32).reshape(B, N)
    Wqkv = np.ascontiguousarray(np.asarray(Wqkv, dtype=np.float32))
    bqkv = np.ascontiguousarray(np.asarray(bqkv, dtype=np.float32))
    Wproj = np.ascontiguousarray(np.asarray(Wproj, dtype=np.float32))
    bproj = np.ascontiguousarray(np.asarray(bproj, dtype=np.float32))

    npdt = ml_dtypes.bfloat16
    # permute tokens per batch: kept (policy=1) first, stable order
    perms = np.empty((B, N), dtype=np.int64)
    for bi in range(B):
        perms[bi] = np.argsort(-policy[bi], kind="stable")
        assert policy[bi].sum() <= MKEY, "kept tokens exceed compacted keys"
    xp = np.take_along_axis(x, perms[:, :, None], axis=1)
    pp = np.take_along_axis(policy, perms, axis=1)

    xpad = np.zeros((B, NPAD, C), dtype=np.float32)
    xpad[:, :N, :] = xp
    # feature-major pre-transpose: [B, NF, 128, NPAD]
    xT = np.ascontiguousarray(
        xpad.transpose(0, 2, 1).reshape(B, NF, 128, NPAD)
    ).astype(npdt)
    Wqkv = Wqkv.astype(npdt)
    Wproj = Wproj.astype(npdt)
    maskb = np.where(pp[:, :MKEY] > 0.5, 0.0, MASK_NEG).astype(np.float32)

    ones_hd = np.zeros((C, H), dtype=np.float32)
    for h in range(H):
        ones_hd[h * DH : (h + 1) * DH, h] = 1.0
    ones_hd = ones_hd.astype(npdt)

    in_maps = []
    for c in range(NCORES):
        b0 = c * BB
        in_maps.append(
            {
                "x": xT[b0 : b0 + BB],
                "maskb": maskb[b0 : b0 + BB],
                "wqkv": Wqkv,
                "wproj": Wproj,
                "bqkv": bqkv,
                "bproj": bproj,
                "ones_hd": ones_hd,
            }
        )
    return in_maps, perms


def run(inputs, trace=False):
    """Run on hardware; returns (output [B,N,C], BassKernelResults)."""
    from concourse.bass_utils import run_bass_kernel_spmd

    nc = _get_nc()
    in_maps, perms = _make_in_maps(**inputs)
    res = run_bass_kernel_spmd(
        nc, in_maps, core_ids=list(range(NCORES)), trace=trace
    )
    out = np.empty((B, N, C), dtype=np.float32)
    for c in range(NCORES):
        o = res.results[c]["out"].reshape(BB, N, C)
        for i in range(BB):
            bi = c * BB + i
            out[bi, perms[bi]] = o[i]
    return out, res


def kernel(x, policy, Wqkv, bqkv, Wproj, bproj):
    out, _ = run(
        dict(x=x, policy=policy, Wqkv=Wqkv, bqkv=bqkv, Wproj=Wproj, bproj=bproj)
    )
    return out


# revision 15
# speedup vs baseline: 1.0580x; 1.0580x over previous
"""Policy-masked multi-head attention block (ViT-style) on 8 TRN2 NeuronCores.

Sharding: data-parallel over batch. B=16 -> 2 batches per core, no collectives.

Key optimization vs naive: tokens are permuted host-side so kept tokens
(policy=1) come first. Kept count m ~ Binomial(577, .5) <= 384 w.p.
1-1e-15, so attention KEYS are compacted to the first 384 tokens (3 tiles
of 128 instead of 5): logits, exp and AV shrink by 40%. Dropped queries
(q >= m) keep their self-attention term:
  - q in [m, 384): self is among the (masked) keys; restored by a
    per-tile diagonal max  E <- max(E, ident * d)  (d = exp(SCALE*q.k)).
  - q in [384, 577): handled by two extra AV accumulation steps with a
    diagonal moving matrix D_t = ident * d (also supplies the ones-row
    denominator contribution).
The output is un-permuted host-side.

Math (per batch, matches reference up to O(1e-3) in bf16):
  qkv = x @ Wqkv + bqkv ; q,k,v per head (Dh=64)
  E[j,q] = exp(SCALE*z[j,q] + mb_j)   (mb_j = -1e4 for dropped keys)
  num[d,q] = sum_j v[j,d]*E[j,q]  (+ diag terms)   den[q] = ones-row
  attn_out[q,d] = num[d,q]/den[q] ;  out = attn_out @ Wproj + bproj
No max-subtraction: |SCALE*z| is O(1) for this input distribution.

Layout: keys-on-partitions / queries-on-free; the policy mask is a
per-partition activation bias; the key-sum (denominator) rides the AV
matmul as a ones column of V; E feeds the AV matmul directly as the
moving operand. Even/odd heads of a pair are processed interleaved
(separate PSUM accumulators) so the PE never waits for a deferred pass.
"""

import os
import ml_dtypes
import numpy as np

import concourse.bass as bass
import concourse.bacc as bacc
import concourse.mybir as mybir
import concourse.tile as tile
from concourse.masks import make_identity

# problem constants (hardcoded per contract)
B = 16
N = 577
C = 768
H = 12
DH = 64
SCALE = DH ** -0.5
EPS = 1e-6
NCORES = 8
BB = B // NCORES          # batches per core
NPAD = 640                # tokens padded to 5*128
NT = NPAD // 128          # 5 token tiles
NKT = 3                   # compacted key tiles (384 keys)
MKEY = NKT * 128          # 384
NF = C // 128             # 6 feature tiles
F32 = mybir.dt.float32
BF16 = mybir.dt.bfloat16
DT = BF16
SCALE_EXP = SCALE

MASK_NEG = -1.0e4
Q1, Q2 = 320, N - 320     # query chunks for 577-wide streams (QK proj)


def build_kernel():
    nc = bacc.Bacc()

    x_d = nc.declare_dram_parameter("x", [BB, NF, 128, NPAD], DT, isOutput=False)
    mb_d = nc.declare_dram_parameter("maskb", [BB, MKEY], F32, isOutput=False)
    wqkv_d = nc.declare_dram_parameter("wqkv", [C, 3 * C], DT, isOutput=False)
    wproj_d = nc.declare_dram_parameter("wproj", [C, C], DT, isOutput=False)
    bqkv_d = nc.declare_dram_parameter("bqkv", [3 * C], F32, isOutput=False)
    bproj_d = nc.declare_dram_parameter("bproj", [C], F32, isOutput=False)
    onehd_d = nc.declare_dram_parameter("ones_hd", [C, H], DT, isOutput=False)
    out_d = nc.declare_dram_parameter("out", [BB * N, C], F32, isOutput=True)

    with tile.TileContext(nc) as tc:
        with (
            tc.tile_pool(name="singles", bufs=1) as singles,
            tc.tile_pool(name="pbatch", bufs=2) as pb,
            tc.tile_pool(name="pe", bufs=2) as pe_pool,
            tc.tile_pool(name="pnum", bufs=2) as pnum,
            tc.tile_pool(name="psmall", bufs=4) as psmall,
            tc.tile_pool(name="ptrans", bufs=2) as ptrans,
            tc.tile_pool(name="pout", bufs=2) as pout,
            tc.tile_pool(name="ppA", bufs=2, space="PSUM") as ppA,
            tc.tile_pool(name="ppN", bufs=1, space="PSUM") as ppN,
        ):
            # ---- constants ----
            wqkv_sb = singles.tile([128, NF, 3 * C], DT)
            for f in range(NF):
                nc.sync.dma_start(
                    wqkv_sb[:, f, :],
                    wqkv_d.rearrange("(f p) m -> f p m", p=128)[f],
                )
            wproj_sb = singles.tile([128, NF, C], DT)
            for f in range(NF):
                nc.scalar.dma_start(
                    wproj_sb[:, f, :],
                    wproj_d.rearrange("(f p) m -> f p m", p=128)[f],
                )
            bqkv_sb = singles.tile([128, 2 * NF], F32)  # q,k feature bias chunks
            nc.scalar.dma_start(
                bqkv_sb[:, :],
                bqkv_d[0 : 2 * C].rearrange("(m p) -> p m", p=128),
            )
            # v-bias and proj-bias broadcast across all 128 partitions
            bv_bc = singles.tile([128, C], F32)
            nc.gpsimd.dma_start(
                out=bv_bc[:, :], in_=bqkv_d[2 * C : 3 * C].partition_broadcast(128)
            )
            bproj_bc = singles.tile([128, C], F32)
            nc.gpsimd.dma_start(
                out=bproj_bc[:, :], in_=bproj_d[:].partition_broadcast(128)
            )
            onehd_sb = singles.tile([128, NF, H], DT)
            nc.scalar.dma_start(
                onehd_sb[:, :, :],
                onehd_d.rearrange("(f p) h -> p f h", p=128),
            )
            ident_f32 = singles.tile([128, 128], F32)
            make_identity(nc, ident_f32)
            ident = singles.tile([128, 128], DT)
            nc.vector.tensor_copy(ident[:, :], ident_f32[:, :])
            ones60 = singles.tile([128, NT * H], F32)
            nc.vector.memset(ones60, 1.0)

            for b in range(BB):
                # ---- load mask + x (transposed to feature-major) ----
                mb_sb = psmall.tile([128, NKT], F32, tag="mb_sb")
                nc.sync.dma_start(
                    mb_sb[:, :], mb_d[b].rearrange("(t p) -> p t", p=128)
                )
                xT = pb.tile([128, NF, NPAD], DT, tag="xT")
                for f in range(NF):
                    nc.sync.dma_start(xT[:, f, :], x_d[b, f])

                # ---- QKV: q,k sections feature-major (577 cols only) ----
                qk_sb = pb.tile([128, 2 * NF, NPAD], DT, tag="qk_sb")
                nc.vector.memset(qk_sb[:, :, N:NPAD], 0.0)
                for m in range(2 * NF):
                    ps = ppA.tile([128, 2, 512], F32, tag="ppA")
                    for f in range(NF):
                        nc.tensor.matmul(
                            ps[:, 0, 0:Q1],
                            wqkv_sb[:, f, m * 128 : (m + 1) * 128],
                            xT[:, f, 0:Q1],
                            start=(f == 0),
                            stop=(f == NF - 1),
                        )
                        nc.tensor.matmul(
                            ps[:, 1, 0:Q2],
                            wqkv_sb[:, f, m * 128 : (m + 1) * 128],
                            xT[:, f, Q1:N],
                            start=(f == 0),
                            stop=(f == NF - 1),
                        )
                    if m % 2 == 0:
                        nc.vector.tensor_scalar_add(
                            qk_sb[:, m, 0:Q1], ps[:, 0, 0:Q1],
                            bqkv_sb[:, m : m + 1],
                        )
                        nc.vector.tensor_scalar_add(
                            qk_sb[:, m, Q1:N], ps[:, 1, 0:Q2],
                            bqkv_sb[:, m : m + 1],
                        )
                    else:
                        nc.scalar.activation(
                            qk_sb[:, m, 0:Q1], ps[:, 0, 0:Q1],
                            mybir.ActivationFunctionType.Identity,
                            bias=bqkv_sb[:, m : m + 1],
                        )
                        nc.scalar.activation(
                            qk_sb[:, m, Q1:N], ps[:, 1, 0:Q2],
                            mybir.ActivationFunctionType.Identity,
                            bias=bqkv_sb[:, m : m + 1],
                        )

                # ---- QKV: v section token-major, per-head layout, ones col
                v_sb = pb.tile([128, NT, H, DH + 1], DT, tag="v_sb")
                for t in range(NT):
                    ps = ppA.tile([128, 2, 512], F32, tag="ppA")
                    for f in range(NF):
                        for n0, n1 in ((0, 512), (512, 768)):
                            nc.tensor.matmul(
                                ps.rearrange("p s q -> p (s q)")[:, n0:n1],
                                xT[:, f, t * 128 : (t + 1) * 128],
                                wqkv_sb[:, f, 2 * C + n0 : 2 * C + n1],
                                start=(f == 0),
                                stop=(f == NF - 1),
                            )
                    nc.vector.tensor_tensor(
                        v_sb[:, t, :, 0:DH],
                        ps.rearrange("p s q -> p (s q)")[:, 0:C].rearrange(
                            "p (h d) -> p h d", h=H
                        ),
                        bv_bc.rearrange("p (h d) -> p h d", h=H),
                        mybir.AluOpType.add,
                    )
                nc.vector.tensor_copy(
                    v_sb[:, :, :, DH],
                    ones60.rearrange("p (t h) -> p t h", t=NT),
                )

                # ---- diagonal logits for all heads: d[h, j] = exp(s*q_j.k_j)
                psz = ppN.tile([128, 2, 512], F32, tag="nume")
                for f in range(NF):
                    qkel = ptrans.tile([128, N], DT, tag="qkel")
                    nc.vector.tensor_tensor(
                        qkel[:, :],
                        qk_sb[:, f, 0:N],
                        qk_sb[:, NF + f, 0:N],
                        mybir.AluOpType.mult,
                    )
                    nc.tensor.matmul(
                        psz[0:H, 0, 0:Q1],
                        onehd_sb[:, f, :],
                        qkel[:, 0:Q1],
                        start=(f == 0),
                        stop=(f == NF - 1),
                    )
                    nc.tensor.matmul(
                        psz[0:H, 1, 0:Q2],
                        onehd_sb[:, f, :],
                        qkel[:, Q1:N],
                        start=(f == 0),
                        stop=(f == NF - 1),
                    )
                d_all = psmall.tile([H, NPAD], F32, tag="d_all")
                nc.scalar.activation(
                    d_all[:, 0:Q1], psz[0:H, 0, 0:Q1],
                    mybir.ActivationFunctionType.Exp, scale=SCALE_EXP,
                )
                nc.scalar.activation(
                    d_all[:, Q1:N], psz[0:H, 1, 0:Q2],
                    mybir.ActivationFunctionType.Exp, scale=SCALE_EXP,
                )
                # keep pad region finite: D rows for pad tokens become 0
                nc.vector.memset(d_all[:, N:NPAD], 0.0)
                d_t = psmall.tile([128, NT, H], F32, tag="d_t")
                for t in range(NT):
                    pst_f = ppN.tile([128, 128], F32, tag="numo")
                    nc.tensor.transpose(
                        pst_f[:, 0:H],
                        d_all[:, t * 128 : (t + 1) * 128],
                        ident_f32[0:H, 0:H],
                    )
                    nc.vector.tensor_copy(d_t[:, t, :], pst_f[:, 0:H])

                # ---- attention, head pairs, software-pipelined emission ----
                # Per hp: logits(t)/exp(t) run one step ahead of AV(t); the
                # num->token-major tail of hp-1 is emitted inside hp's stream
                # so the PE never waits on the PSUM ring or cross-engine
                # copies. Diagonal D tiles are built lazily per hp.
                attn_sb = pb.tile([128, NT, H, DH], DT, tag="attn_sb")
                Dd = pb.tile([128, 2, H, 128], DT, tag="Dd")

                def emit_logits_exp(hp, t):
                    he, ho = 2 * hp, 2 * hp + 1
                    st_e = ppA.tile([128, 2, 512], F32, tag="ppA", name="st_e")
                    st_o = ppA.tile([128, 2, 512], F32, tag="ppA", name="st_o")
                    for s in range(2):
                        nc.tensor.matmul(
                            st_e[:, s, 0:320],
                            qk_sb[0:DH, NF + hp, t * 128 : (t + 1) * 128],
                            qk_sb[0:DH, hp, s * 320 : s * 320 + 320],
                            start=True,
                            stop=True,
                        )
                    for s in range(2):
                        nc.tensor.matmul(
                            st_o[:, s, 0:320],
                            qk_sb[DH:128, NF + hp, t * 128 : (t + 1) * 128],
                            qk_sb[DH:128, hp, s * 320 : s * 320 + 320],
                            start=True,
                            stop=True,
                        )
                    e_e = pe_pool.tile([128, NPAD], DT, tag="e_e", name="e_e")
                    e_o = pe_pool.tile([128, NPAD], DT, tag="e_o", name="e_o")
                    nc.scalar.activation(
                        e_e.rearrange("p (s q) -> p s q", s=2),
                        st_e[:, :, 0:320],
                        mybir.ActivationFunctionType.Exp,
                        bias=mb_sb[:, t : t + 1], scale=SCALE_EXP,
                    )
                    nc.scalar.activation(
                        e_o.rearrange("p (s q) -> p s q", s=2),
                        st_o[:, :, 0:320],
                        mybir.ActivationFunctionType.Exp,
                        bias=mb_sb[:, t : t + 1], scale=SCALE_EXP,
                    )
                    nc.vector.scalar_tensor_tensor(
                        out=e_e[:, t * 128 : (t + 1) * 128],
                        in0=ident,
                        scalar=d_t[:, t, he : he + 1],
                        in1=e_e[:, t * 128 : (t + 1) * 128],
                        op0=mybir.AluOpType.mult,
                        op1=mybir.AluOpType.max,
                    )
                    nc.vector.scalar_tensor_tensor(
                        out=e_o[:, t * 128 : (t + 1) * 128],
                        in0=ident,
                        scalar=d_t[:, t, ho : ho + 1],
                        in1=e_o[:, t * 128 : (t + 1) * 128],
                        op0=mybir.AluOpType.mult,
                        op1=mybir.AluOpType.max,
                    )
                    return e_e, e_o

                def emit_av(hp, t, nume, numo, e_e, e_o):
                    he, ho = 2 * hp, 2 * hp + 1
                    if t == NKT - 1:
                        # tail-query self terms: diagonal D tiles join the
                        # AV group before its final (stop) matmuls
                        for h, num in ((he, nume), (ho, numo)):
                            nc.tensor.matmul(
                                num[:, 1, 64:192],
                                v_sb[:, 3, h, :],
                                Dd[:, 0, h, :],
                                start=False, stop=False,
                                skip_group_check=True,
                            )
                            nc.tensor.matmul(
                                num[:, 1, 192:257],
                                v_sb[:, 4, h, :],
                                Dd[:, 1, h, 0:65],
                                start=False, stop=False,
                                skip_group_check=True,
                            )
                    for s in range(2):
                        nc.tensor.matmul(
                            nume[:, s, 0:320],
                            v_sb[:, t, he, :],
                            e_e[:, s * 320 : s * 320 + 320],
                            start=(t == 0),
                            stop=(t == NKT - 1),
                        )
                    for s in range(2):
                        nc.tensor.matmul(
                            numo[:, s, 0:320],
                            v_sb[:, t, ho, :],
                            e_o[:, s * 320 : s * 320 + 320],
                            start=(t == 0),
                            stop=(t == NKT - 1),
                        )

                def make_tail(hp, num_sbs):
                    def tail():
                        he, ho = 2 * hp, 2 * hp + 1
                        for par, h in ((0, he), (1, ho)):
                            num_sb = num_sbs[par]
                            nt_all = ppN.tile(
                                [128, NT, DH + 2], DT,
                                tag=("nume" if par == 0 else "numo"),
                                name="nt_all",
                            )
                            for t in range(NT):
                                nc.tensor.transpose(
                                    nt_all[:, t, 0 : DH + 1],
                                    num_sb[:, t * 128 : (t + 1) * 128],
                                    ident[0 : DH + 1, 0 : DH + 1],
                                )
                            r = psmall.tile([128, NT], F32, tag="r", name="r")
                            nc.vector.reciprocal(r[:, :], nt_all[:, :, DH])
                            nc.vector.tensor_tensor(
                                attn_sb[:, :, h, :],
                                nt_all[:, :, 0:DH],
                                r[:, :, None].to_broadcast([128, NT, DH]),
                                mybir.AluOpType.mult,
                            )
                    return tail

                pending_tail = None
                for hp in range(H // 2):
                    he, ho = 2 * hp, 2 * hp + 1
                    e0 = emit_logits_exp(hp, 0)
                    e1 = emit_logits_exp(hp, 1)
                    if pending_tail is not None:
                        pending_tail()
                    # D tiles for this hp (DVE, queued behind the tail's
                    # normalize so AV(t0)'s ring dependency clears first)
                    for h in (he, ho):
                        for ti in range(2):
                            nc.vector.tensor_scalar_mul(
                                Dd[:, ti, h, :],
                                ident[:, :],
                                d_t[:, NKT + ti, h : h + 1],
                            )
                    nume = ppN.tile([DH + 1, 2, 512], F32, tag="nume")
                    numo = ppN.tile([DH + 1, 2, 512], F32, tag="numo")
                    emit_av(hp, 0, nume, numo, *e0)
                    e2 = emit_logits_exp(hp, 2)
                    emit_av(hp, 1, nume, numo, *e1)
                    emit_av(hp, 2, nume, numo, *e2)
                    num_sbs = []
                    for par, num in ((0, nume), (1, numo)):
                        num_sb = pnum.tile(
                            [DH + 1, NPAD], DT, tag=f"num{par}", name="num_sb"
                        )
                        if par == 0:
                            nc.scalar.activation(
                                num_sb.rearrange("p (s q) -> p s q", s=2),
                                num[:, :, 0:320],
                                mybir.ActivationFunctionType.Identity,
                            )
                        else:
                            nc.vector.tensor_copy(
                                num_sb.rearrange("p (s q) -> p s q", s=2),
                                num[:, :, 0:320],
                            )
                        num_sbs.append(num_sb)
                    pending_tail = make_tail(hp, num_sbs)
                pending_tail()

                # ---- proj: transposes of tile t overlap matmuls of t-1 ----
                def emit_proj_mms(pso, aT6, t):
                    for f in range(NF):
                        for n0, n1 in ((0, 512), (512, 768)):
                            nc.tensor.matmul(
                                pso.rearrange("p s q -> p (s q)")[:, n0:n1],
                                aT6[:, f, :],
                                wproj_sb[:, f, n0:n1],
                                start=(f == 0),
                                stop=(f == NF - 1),
                            )
                    o_sb = pout.tile([128, C], F32, tag="o_sb", name="o_sb")
                    nc.vector.tensor_tensor(
                        o_sb[:, :],
                        pso.rearrange("p s q -> p (s q)")[:, 0:C],
                        bproj_bc[:, :],
                        mybir.AluOpType.add,
                    )
                    rows = 128 if t < NT - 1 else N - 4 * 128
                    nc.sync.dma_start(
                        out_d[b * N + t * 128 : b * N + t * 128 + rows, :],
                        o_sb[0:rows, :],
                    )

                prev_proj = None
                for t in range(NT):
                    pso = ppA.tile([128, 2, 512], F32, tag="ppA", name="pso")
                    aT6 = ptrans.tile([128, NF, 128], DT, tag="aT", name="aT6")
                    for f in range(NF):
                        pst = ppN.tile(
                            [128, 128], DT,
                            tag=("nume" if f % 2 == 0 else "numo"), name="pst",
                        )
                        nc.tensor.transpose(
                            pst[:, 0:128],
                            attn_sb.rearrange("p t h d -> p t (h d)")[
                                :, t, f * 128 : (f + 1) * 128
                            ],
                            ident,
                        )
                        if f % 2 == 0:
                            nc.scalar.activation(
                                aT6[:, f, :], pst[:, 0:128],
                                mybir.ActivationFunctionType.Identity,
                            )
                        else:
                            nc.vector.tensor_copy(aT6[:, f, :], pst[:, 0:128])
                    if prev_proj is not None:
                        emit_proj_mms(*prev_proj)
                    prev_proj = (pso, aT6, t)
                emit_proj_mms(*prev_proj)
    nc.finalize()
    return nc


_NC_CACHE = None


def _get_nc():
    global _NC_CACHE
    if _NC_CACHE is None:
        _NC_CACHE = build_kernel()
    return _NC_CACHE


def _make_in_maps(x, policy, Wqkv, bqkv, Wproj, bproj):
    x = np.ascontiguousarray(np.asarray(x, dtype=np.float32))
    policy = np.asarray(policy, dtype=np.float32).reshape(B, N)
    Wqkv = np.ascontiguousarray(np.asarray(Wqkv, dtype=np.float32))
    bqkv = np.ascontiguousarray(np.asarray(bqkv, dtype=np.float32))
    Wproj = np.ascontiguousarray(np.asarray(Wproj, dtype=np.float32))
    bproj = np.ascontiguousarray(np.asarray(bproj, dtype=np.float32))

    npdt = ml_dtypes.bfloat16
    # permute tokens per batch: kept (policy=1) first, stable order
    perms = np.empty((B, N), dtype=np.int64)
    for bi in range(B):
        perms[bi] = np.argsort(-policy[bi], kind="stable")
        assert policy[bi].sum() <= MKEY, "kept tokens exceed compacted keys"
    xp = np.take_along_axis(x, perms[:, :, None], axis=1)
    pp = np.take_along_axis(policy, perms, axis=1)

    xpad = np.zeros((B, NPAD, C), dtype=np.float32)
    xpad[:, :N, :] = xp
    # feature-major pre-transpose: [B, NF, 128, NPAD]
    xT = np.ascontiguousarray(
        xpad.transpose(0, 2, 1).reshape(B, NF, 128, NPAD)
    ).astype(npdt)
    Wqkv = Wqkv.astype(npdt)
    Wproj = Wproj.astype(npdt)
    maskb = np.where(pp[:, :MKEY] > 0.5, 0.0, MASK_NEG).astype(np.float32)

    ones_hd = np.zeros((C, H), dtype=np.float32)
    for h in range(H):
        ones_hd[h * DH : (h + 1) * DH, h] = 1.0
    ones_hd = ones_hd.astype(npdt)

    in_maps = []
    for c in range(NCORES):
        b0 = c * BB
        in_maps.append(
            {
                "x": xT[b0 : b0 + BB],
                "maskb": maskb[b0 : b0 + BB],
                "wqkv": Wqkv,
                "wproj": Wproj,
                "bqkv": bqkv,
                "bproj": bproj,
                "ones_hd": ones_hd,
            }
        )
    return in_maps, perms


def run(inputs, trace=False):
    """Run on hardware; returns (output [B,N,C], BassKernelResults)."""
    from concourse.bass_utils import run_bass_kernel_spmd

    nc = _get_nc()
    in_maps, perms = _make_in_maps(**inputs)
    res = run_bass_kernel_spmd(
        nc, in_maps, core_ids=list(range(NCORES)), trace=trace
    )
    out = np.empty((B, N, C), dtype=np.float32)
    for c in range(NCORES):
        o = res.results[c]["out"].reshape(BB, N, C)
        for i in range(BB):
            bi = c * BB + i
            out[bi, perms[bi]] = o[i]
    return out, res


def kernel(x, policy, Wqkv, bqkv, Wproj, bproj):
    out, _ = run(
        dict(x=x, policy=policy, Wqkv=Wqkv, bqkv=bqkv, Wproj=Wproj, bproj=bproj)
    )
    return out


# revision 16
# speedup vs baseline: 1.0714x; 1.0127x over previous
"""Policy-masked multi-head attention block (ViT-style) on 8 TRN2 NeuronCores.

Sharding: data-parallel over batch. B=16 -> 2 batches per core, no collectives.

Key optimization vs naive: tokens are permuted host-side so kept tokens
(policy=1) come first. Kept count m ~ Binomial(577, .5) <= 384 w.p.
1-1e-15, so attention KEYS are compacted to the first 384 tokens (3 tiles
of 128 instead of 5): logits, exp and AV shrink by 40%. Dropped queries
(q >= m) keep their self-attention term:
  - q in [m, 384): self is among the (masked) keys; restored by a
    per-tile diagonal max  E <- max(E, ident * d)  (d = exp(SCALE*q.k)).
  - q in [384, 577): handled by two extra AV accumulation steps with a
    diagonal moving matrix D_t = ident * d (also supplies the ones-row
    denominator contribution).
The output is un-permuted host-side.

Math (per batch, matches reference up to O(1e-3) in bf16):
  qkv = x @ Wqkv + bqkv ; q,k,v per head (Dh=64)
  E[j,q] = exp(SCALE*z[j,q] + mb_j)   (mb_j = -1e4 for dropped keys)
  num[d,q] = sum_j v[j,d]*E[j,q]  (+ diag terms)   den[q] = ones-row
  attn_out[q,d] = num[d,q]/den[q] ;  out = attn_out @ Wproj + bproj
No max-subtraction: |SCALE*z| is O(1) for this input distribution.

Layout: keys-on-partitions / queries-on-free; the policy mask is a
per-partition activation bias; the key-sum (denominator) rides the AV
matmul as a ones column of V; E feeds the AV matmul directly as the
moving operand. Even/odd heads of a pair are processed interleaved
(separate PSUM accumulators) so the PE never waits for a deferred pass.
"""

import os
import ml_dtypes
import numpy as np

import concourse.bass as bass
import concourse.bacc as bacc
import concourse.mybir as mybir
import concourse.tile as tile
from concourse.masks import make_identity

# problem constants (hardcoded per contract)
B = 16
N = 577
C = 768
H = 12
DH = 64
SCALE = DH ** -0.5
EPS = 1e-6
NCORES = 8
BB = B // NCORES          # batches per core
NPAD = 640                # tokens padded to 5*128
NT = NPAD // 128          # 5 token tiles
NKT = 3                   # compacted key tiles (384 keys)
MKEY = NKT * 128          # 384
NF = C // 128             # 6 feature tiles
F32 = mybir.dt.float32
BF16 = mybir.dt.bfloat16
DT = BF16
SCALE_EXP = SCALE

MASK_NEG = -1.0e4
Q1, Q2 = 320, N - 320     # query chunks for 577-wide streams (QK proj)


def build_kernel():
    nc = bacc.Bacc()

    x_d = nc.declare_dram_parameter("x", [BB, NF, 128, NPAD], DT, isOutput=False)
    mb_d = nc.declare_dram_parameter("maskb", [BB, MKEY], F32, isOutput=False)
    wqkv_d = nc.declare_dram_parameter("wqkv", [C, 3 * C], DT, isOutput=False)
    wproj_d = nc.declare_dram_parameter("wproj", [C, C], DT, isOutput=False)
    bqkv_d = nc.declare_dram_parameter("bqkv", [3 * C], F32, isOutput=False)
    bproj_d = nc.declare_dram_parameter("bproj", [C], F32, isOutput=False)
    onehd_d = nc.declare_dram_parameter("ones_hd", [C, H], DT, isOutput=False)
    out_d = nc.declare_dram_parameter("out", [BB * N, C], F32, isOutput=True)

    with tile.TileContext(nc) as tc:
        with (
            tc.tile_pool(name="singles", bufs=1) as singles,
            tc.tile_pool(name="pbatch", bufs=2) as pb,
            tc.tile_pool(name="pe", bufs=2) as pe_pool,
            tc.tile_pool(name="pnum", bufs=2) as pnum,
            tc.tile_pool(name="psmall", bufs=4) as psmall,
            tc.tile_pool(name="ptrans", bufs=2) as ptrans,
            tc.tile_pool(name="pout", bufs=2) as pout,
            tc.tile_pool(name="ppA", bufs=2, space="PSUM") as ppA,
            tc.tile_pool(name="ppN", bufs=1, space="PSUM") as ppN,
        ):
            # ---- constants ----
            wqkv_sb = singles.tile([128, NF, 3 * C], DT)
            for f in range(NF):
                nc.sync.dma_start(
                    wqkv_sb[:, f, :],
                    wqkv_d.rearrange("(f p) m -> f p m", p=128)[f],
                )
            wproj_sb = singles.tile([128, NF, C], DT)
            for f in range(NF):
                nc.scalar.dma_start(
                    wproj_sb[:, f, :],
                    wproj_d.rearrange("(f p) m -> f p m", p=128)[f],
                )
            bqkv_sb = singles.tile([128, 2 * NF], F32)  # q,k feature bias chunks
            nc.scalar.dma_start(
                bqkv_sb[:, :],
                bqkv_d[0 : 2 * C].rearrange("(m p) -> p m", p=128),
            )
            # v-bias and proj-bias broadcast across all 128 partitions
            bv_bc = singles.tile([128, C], F32)
            nc.gpsimd.dma_start(
                out=bv_bc[:, :], in_=bqkv_d[2 * C : 3 * C].partition_broadcast(128)
            )
            bproj_bc = singles.tile([128, C], F32)
            nc.gpsimd.dma_start(
                out=bproj_bc[:, :], in_=bproj_d[:].partition_broadcast(128)
            )
            onehd_sb = singles.tile([128, NF, H], DT)
            nc.scalar.dma_start(
                onehd_sb[:, :, :],
                onehd_d.rearrange("(f p) h -> p f h", p=128),
            )
            ident_f32 = singles.tile([128, 128], F32)
            make_identity(nc, ident_f32)
            ident = singles.tile([128, 128], DT)
            nc.vector.tensor_copy(ident[:, :], ident_f32[:, :])
            ones60 = singles.tile([128, NT * H], F32)
            nc.vector.memset(ones60, 1.0)

            for b in range(BB):
                # ---- load mask + x (transposed to feature-major) ----
                mb_sb = psmall.tile([128, NKT], F32, tag="mb_sb")
                nc.sync.dma_start(
                    mb_sb[:, :], mb_d[b].rearrange("(t p) -> p t", p=128)
                )
                xT = pb.tile([128, NF, NPAD], DT, tag="xT")
                for f in range(NF):
                    nc.sync.dma_start(xT[:, f, :], x_d[b, f])

                # ---- QKV: q,k sections feature-major (577 cols only) ----
                qk_sb = pb.tile([128, 2 * NF, NPAD], DT, tag="qk_sb")
                nc.vector.memset(qk_sb[:, :, N:NPAD], 0.0)
                for m in range(2 * NF):
                    ps = ppA.tile([128, 2, 512], F32, tag="ppA")
                    for f in range(NF):
                        nc.tensor.matmul(
                            ps[:, 0, 0:Q1],
                            wqkv_sb[:, f, m * 128 : (m + 1) * 128],
                            xT[:, f, 0:Q1],
                            start=(f == 0),
                            stop=(f == NF - 1),
                        )
                        nc.tensor.matmul(
                            ps[:, 1, 0:Q2],
                            wqkv_sb[:, f, m * 128 : (m + 1) * 128],
                            xT[:, f, Q1:N],
                            start=(f == 0),
                            stop=(f == NF - 1),
                        )
                    if m % 2 == 0:
                        nc.vector.tensor_scalar_add(
                            qk_sb[:, m, 0:Q1], ps[:, 0, 0:Q1],
                            bqkv_sb[:, m : m + 1],
                        )
                        nc.vector.tensor_scalar_add(
                            qk_sb[:, m, Q1:N], ps[:, 1, 0:Q2],
                            bqkv_sb[:, m : m + 1],
                        )
                    else:
                        nc.scalar.activation(
                            qk_sb[:, m, 0:Q1], ps[:, 0, 0:Q1],
                            mybir.ActivationFunctionType.Identity,
                            bias=bqkv_sb[:, m : m + 1],
                        )
                        nc.scalar.activation(
                            qk_sb[:, m, Q1:N], ps[:, 1, 0:Q2],
                            mybir.ActivationFunctionType.Identity,
                            bias=bqkv_sb[:, m : m + 1],
                        )

                # ---- QKV: v section token-major, per-head layout, ones col
                v_sb = pb.tile([128, NT, H, DH + 1], DT, tag="v_sb")
                for t in range(NT):
                    ps = ppA.tile([128, 2, 512], F32, tag="ppA")
                    for f in range(NF):
                        for n0, n1 in ((0, 512), (512, 768)):
                            nc.tensor.matmul(
                                ps.rearrange("p s q -> p (s q)")[:, n0:n1],
                                xT[:, f, t * 128 : (t + 1) * 128],
                                wqkv_sb[:, f, 2 * C + n0 : 2 * C + n1],
                                start=(f == 0),
                                stop=(f == NF - 1),
                            )
                    nc.vector.tensor_tensor(
                        v_sb[:, t, :, 0:DH],
                        ps.rearrange("p s q -> p (s q)")[:, 0:C].rearrange(
                            "p (h d) -> p h d", h=H
                        ),
                        bv_bc.rearrange("p (h d) -> p h d", h=H),
                        mybir.AluOpType.add,
                    )
                nc.vector.tensor_copy(
                    v_sb[:, :, :, DH],
                    ones60.rearrange("p (t h) -> p t h", t=NT),
                )

                # ---- diagonal logits for all heads: d[h, j] = exp(s*q_j.k_j)
                psz = ppN.tile([128, 2, 512], F32, tag="nume")
                for f in range(NF):
                    qkel = ptrans.tile([128, N], DT, tag="qkel")
                    nc.vector.tensor_tensor(
                        qkel[:, :],
                        qk_sb[:, f, 0:N],
                        qk_sb[:, NF + f, 0:N],
                        mybir.AluOpType.mult,
                    )
                    nc.tensor.matmul(
                        psz[0:H, 0, 0:Q1],
                        onehd_sb[:, f, :],
                        qkel[:, 0:Q1],
                        start=(f == 0),
                        stop=(f == NF - 1),
                    )
                    nc.tensor.matmul(
                        psz[0:H, 1, 0:Q2],
                        onehd_sb[:, f, :],
                        qkel[:, Q1:N],
                        start=(f == 0),
                        stop=(f == NF - 1),
                    )
                d_all = psmall.tile([H, NPAD], F32, tag="d_all")
                nc.scalar.activation(
                    d_all[:, 0:Q1], psz[0:H, 0, 0:Q1],
                    mybir.ActivationFunctionType.Exp, scale=SCALE_EXP,
                )
                nc.scalar.activation(
                    d_all[:, Q1:N], psz[0:H, 1, 0:Q2],
                    mybir.ActivationFunctionType.Exp, scale=SCALE_EXP,
                )
                # keep pad region finite: D rows for pad tokens become 0
                nc.vector.memset(d_all[:, N:NPAD], 0.0)
                d_t = psmall.tile([128, NT, H], F32, tag="d_t")
                for t in range(NT):
                    pst_f = ppN.tile([128, 128], F32, tag="numo")
                    nc.tensor.transpose(
                        pst_f[:, 0:H],
                        d_all[:, t * 128 : (t + 1) * 128],
                        ident_f32[0:H, 0:H],
                    )
                    nc.vector.tensor_copy(d_t[:, t, :], pst_f[:, 0:H])

                # ---- attention, head pairs, software-pipelined emission ----
                # Per hp: logits(t)/exp(t) run one step ahead of AV(t); the
                # num->token-major tail of hp-1 is emitted inside hp's stream
                # so the PE never waits on the PSUM ring or cross-engine
                # copies. Diagonal D tiles are built lazily per hp.
                attn_sb = pb.tile([128, NT, H, DH], DT, tag="attn_sb")
                Dd = pb.tile([128, 2, H, 128], DT, tag="Dd")

                def emit_logits_exp(hp, t):
                    he, ho = 2 * hp, 2 * hp + 1
                    st_e = ppA.tile([128, 2, 512], F32, tag="ppA", name="st_e")
                    st_o = ppA.tile([128, 2, 512], F32, tag="ppA", name="st_o")
                    for s in range(2):
                        nc.tensor.matmul(
                            st_e[:, s, 0:320],
                            qk_sb[0:DH, NF + hp, t * 128 : (t + 1) * 128],
                            qk_sb[0:DH, hp, s * 320 : s * 320 + 320],
                            start=True,
                            stop=True,
                        )
                        nc.tensor.matmul(
                            st_o[:, s, 0:320],
                            qk_sb[DH:128, NF + hp, t * 128 : (t + 1) * 128],
                            qk_sb[DH:128, hp, s * 320 : s * 320 + 320],
                            start=True,
                            stop=True,
                        )
                    e_e = pe_pool.tile([128, NPAD], DT, tag="e_e", name="e_e")
                    e_o = pe_pool.tile([128, NPAD], DT, tag="e_o", name="e_o")
                    nc.scalar.activation(
                        e_e.rearrange("p (s q) -> p s q", s=2),
                        st_e[:, :, 0:320],
                        mybir.ActivationFunctionType.Exp,
                        bias=mb_sb[:, t : t + 1], scale=SCALE_EXP,
                    )
                    nc.scalar.activation(
                        e_o.rearrange("p (s q) -> p s q", s=2),
                        st_o[:, :, 0:320],
                        mybir.ActivationFunctionType.Exp,
                        bias=mb_sb[:, t : t + 1], scale=SCALE_EXP,
                    )
                    nc.vector.scalar_tensor_tensor(
                        out=e_e[:, t * 128 : (t + 1) * 128],
                        in0=ident,
                        scalar=d_t[:, t, he : he + 1],
                        in1=e_e[:, t * 128 : (t + 1) * 128],
                        op0=mybir.AluOpType.mult,
                        op1=mybir.AluOpType.max,
                    )
                    nc.vector.scalar_tensor_tensor(
                        out=e_o[:, t * 128 : (t + 1) * 128],
                        in0=ident,
                        scalar=d_t[:, t, ho : ho + 1],
                        in1=e_o[:, t * 128 : (t + 1) * 128],
                        op0=mybir.AluOpType.mult,
                        op1=mybir.AluOpType.max,
                    )
                    return e_e, e_o

                def emit_av(hp, t, nume, numo, e_e, e_o):
                    he, ho = 2 * hp, 2 * hp + 1
                    if t == NKT - 1:
                        # tail-query self terms: diagonal D tiles join the
                        # AV group before its final (stop) matmuls
                        for h, num in ((he, nume), (ho, numo)):
                            nc.tensor.matmul(
                                num[:, 1, 64:192],
                                v_sb[:, 3, h, :],
                                Dd[:, 0, h, :],
                                start=False, stop=False,
                                skip_group_check=True,
                            )
                            nc.tensor.matmul(
                                num[:, 1, 192:257],
                                v_sb[:, 4, h, :],
                                Dd[:, 1, h, 0:65],
                                start=False, stop=False,
                                skip_group_check=True,
                            )
                    for s in range(2):
                        nc.tensor.matmul(
                            nume[:, s, 0:320],
                            v_sb[:, t, he, :],
                            e_e[:, s * 320 : s * 320 + 320],
                            start=(t == 0),
                            stop=(t == NKT - 1),
                        )
                        nc.tensor.matmul(
                            numo[:, s, 0:320],
                            v_sb[:, t, ho, :],
                            e_o[:, s * 320 : s * 320 + 320],
                            start=(t == 0),
                            stop=(t == NKT - 1),
                        )

                def make_tail(hp, num_sbs):
                    def tail():
                        he, ho = 2 * hp, 2 * hp + 1
                        for par, h in ((0, he), (1, ho)):
                            num_sb = num_sbs[par]
                            nt_all = ppN.tile(
                                [128, NT, DH + 2], DT,
                                tag=("nume" if par == 0 else "numo"),
                                name="nt_all",
                            )
                            for t in range(NT):
                                nc.tensor.transpose(
                                    nt_all[:, t, 0 : DH + 1],
                                    num_sb[:, t * 128 : (t + 1) * 128],
                                    ident[0 : DH + 1, 0 : DH + 1],
                                )
                            r = psmall.tile([128, NT], F32, tag="r", name="r")
                            nc.vector.reciprocal(r[:, :], nt_all[:, :, DH])
                            nc.vector.tensor_tensor(
                                attn_sb[:, :, h, :],
                                nt_all[:, :, 0:DH],
                                r[:, :, None].to_broadcast([128, NT, DH]),
                                mybir.AluOpType.mult,
                            )
                    return tail

                pending_tail = None
                for hp in range(H // 2):
                    he, ho = 2 * hp, 2 * hp + 1
                    e0 = emit_logits_exp(hp, 0)
                    # D tiles for this hp (DVE, queued behind the t0 stt ops)
                    for h in (he, ho):
                        for ti in range(2):
                            nc.vector.tensor_scalar_mul(
                                Dd[:, ti, h, :],
                                ident[:, :],
                                d_t[:, NKT + ti, h : h + 1],
                            )
                    e1 = emit_logits_exp(hp, 1)
                    if pending_tail is not None:
                        pending_tail()
                    nume = ppN.tile([DH + 1, 2, 512], F32, tag="nume")
                    numo = ppN.tile([DH + 1, 2, 512], F32, tag="numo")
                    emit_av(hp, 0, nume, numo, *e0)
                    e2 = emit_logits_exp(hp, 2)
                    emit_av(hp, 1, nume, numo, *e1)
                    emit_av(hp, 2, nume, numo, *e2)
                    num_sbs = []
                    for par, num in ((0, nume), (1, numo)):
                        num_sb = pnum.tile(
                            [DH + 1, NPAD], DT, tag=f"num{par}", name="num_sb"
                        )
                        if par == 0:
                            nc.scalar.activation(
                                num_sb.rearrange("p (s q) -> p s q", s=2),
                                num[:, :, 0:320],
                                mybir.ActivationFunctionType.Identity,
                            )
                        else:
                            nc.vector.tensor_copy(
                                num_sb.rearrange("p (s q) -> p s q", s=2),
                                num[:, :, 0:320],
                            )
                        num_sbs.append(num_sb)
                    pending_tail = make_tail(hp, num_sbs)
                pending_tail()

                # ---- proj: transposes of tile t overlap matmuls of t-1 ----
                def emit_proj_mms(pso, aT6, t):
                    for f in range(NF):
                        for n0, n1 in ((0, 512), (512, 768)):
                            nc.tensor.matmul(
                                pso.rearrange("p s q -> p (s q)")[:, n0:n1],
                                aT6[:, f, :],
                                wproj_sb[:, f, n0:n1],
                                start=(f == 0),
                                stop=(f == NF - 1),
                            )
                    o_sb = pout.tile([128, C], F32, tag="o_sb", name="o_sb")
                    nc.vector.tensor_tensor(
                        o_sb[:, :],
                        pso.rearrange("p s q -> p (s q)")[:, 0:C],
                        bproj_bc[:, :],
                        mybir.AluOpType.add,
                    )
                    rows = 128 if t < NT - 1 else N - 4 * 128
                    nc.sync.dma_start(
                        out_d[b * N + t * 128 : b * N + t * 128 + rows, :],
                        o_sb[0:rows, :],
                    )

                prev_proj = None
                for t in range(NT):
                    pso = ppA.tile([128, 2, 512], F32, tag="ppA", name="pso")
                    aT6 = ptrans.tile([128, NF, 128], DT, tag="aT", name="aT6")
                    for f in range(NF):
                        pst = ppN.tile(
                            [128, 128], DT,
                            tag=("nume" if f % 2 == 0 else "numo"), name="pst",
                        )
                        nc.tensor.transpose(
                            pst[:, 0:128],
                            attn_sb.rearrange("p t h d -> p t (h d)")[
                                :, t, f * 128 : (f + 1) * 128
                            ],
                            ident,
                        )
                        if f % 2 == 0:
                            nc.scalar.activation(
                                aT6[:, f, :], pst[:, 0:128],
                                mybir.ActivationFunctionType.Identity,
                            )
                        else:
                            nc.vector.tensor_copy(aT6[:, f, :], pst[:, 0:128])
                    if prev_proj is not None:
                        emit_proj_mms(*prev_proj)
                    prev_proj = (pso, aT6, t)
                emit_proj_mms(*prev_proj)
    nc.finalize()
    return nc


_NC_CACHE = None


def _get_nc():
    global _NC_CACHE
    if _NC_CACHE is None:
        _NC_CACHE = build_kernel()
    return _NC_CACHE


def _make_in_maps(x, policy, Wqkv, bqkv, Wproj, bproj):
    x = np.ascontiguousarray(np.asarray(x, dtype=np.float32))
    policy = np.asarray(policy, dtype=np.float32).reshape(B, N)
    Wqkv = np.ascontiguousarray(np.asarray(Wqkv, dtype=np.float32))
    bqkv = np.ascontiguousarray(np.asarray(bqkv, dtype=np.float32))
    Wproj = np.ascontiguousarray(np.asarray(Wproj, dtype=np.float32))
    bproj = np.ascontiguousarray(np.asarray(bproj, dtype=np.float32))

    npdt = ml_dtypes.bfloat16
    # permute tokens per batch: kept (policy=1) first, stable order
    perms = np.empty((B, N), dtype=np.int64)
    for bi in range(B):
        perms[bi] = np.argsort(-policy[bi], kind="stable")
        assert policy[bi].sum() <= MKEY, "kept tokens exceed compacted keys"
    xp = np.take_along_axis(x, perms[:, :, None], axis=1)
    pp = np.take_along_axis(policy, perms, axis=1)

    xpad = np.zeros((B, NPAD, C), dtype=np.float32)
    xpad[:, :N, :] = xp
    # feature-major pre-transpose: [B, NF, 128, NPAD]
    xT = np.ascontiguousarray(
        xpad.transpose(0, 2, 1).reshape(B, NF, 128, NPAD)
    ).astype(npdt)
    Wqkv = Wqkv.astype(npdt)
    Wproj = Wproj.astype(npdt)
    maskb = np.where(pp[:, :MKEY] > 0.5, 0.0, MASK_NEG).astype(np.float32)

    ones_hd = np.zeros((C, H), dtype=np.float32)
    for h in range(H):
        ones_hd[h * DH : (h + 1) * DH, h] = 1.0
    ones_hd = ones_hd.astype(npdt)

    in_maps = []
    for c in range(NCORES):
        b0 = c * BB
        in_maps.append(
            {
                "x": xT[b0 : b0 + BB],
                "maskb": maskb[b0 : b0 + BB],
                "wqkv": Wqkv,
                "wproj": Wproj,
                "bqkv": bqkv,
                "bproj": bproj,
                "ones_hd": ones_hd,
            }
        )
    return in_maps, perms


def run(inputs, trace=False):
    """Run on hardware; returns (output [B,N,C], BassKernelResults)."""
    from concourse.bass_utils import run_bass_kernel_spmd

    nc = _get_nc()
    in_maps, perms = _make_in_maps(**inputs)
    res = run_bass_kernel_spmd(
        nc, in_maps, core_ids=list(range(NCORES)), trace=trace
    )
    out = np.empty((B, N, C), dtype=np.float32)
    for c in range(NCORES):
        o = res.results[c]["out"].reshape(BB, N, C)
        for i in range(BB):
            bi = c * BB + i
            out[bi, perms[bi]] = o[i]
    return out, res


def kernel(x, policy, Wqkv, bqkv, Wproj, bproj):
    out, _ = run(
        dict(x=x, policy=policy, Wqkv=Wqkv, bqkv=bqkv, Wproj=Wproj, bproj=bproj)
    )
    return out
